# revision 1
# baseline (speedup 1.0000x reference)
"""NeighborAttention on 8 NeuronCores (node-dim sharded, folded-projection math).

Folded algebra (avoids the reference's per-edge K/V projections entirely):
  Qt[n,h,j]    = sum_d (h_V W_Q^T)[n,h,d] * W_K[(h,d), j]
  logits[n,h,k]= sum_j Qt[n,h,j] * h_E[n,k,j] / sqrt(d)   (+ mask -> NEG_INF)
  attend       = softmax_k(logits) * mask
  agg[n,h,j]   = sum_k attend[n,h,k] * h_E[n,k,j]
  ctx[n,(h,d)] = sum_j W_V[(h,d), j] * agg[n,h,j]
  out          = ctx @ W_O^T
This is ~14x fewer FLOPs than the reference formulation; h_E is read once.
"""
import numpy as np
import jax
import jax.numpy as jnp
from jax.sharding import Mesh, PartitionSpec as P
from jax.experimental.shard_map import shard_map
from functools import partial

B, N, K, F, HID, H, D = 4, 4096, 32, 384, 128, 4, 32
NEG_INF = np.finfo(np.float32).min
_CACHE = {}


def _core_fn(h_V, h_E, mask, W_Q, W_K, W_V, W_O):
    # shapes per core: h_V [B, Ns, HID], h_E [B, Ns, K, F], mask [B, Ns, K]
    Q = jnp.einsum('bnf,if->bni', h_V, W_Q).reshape(*h_V.shape[:2], H, D)
    WKh = W_K.reshape(H, D, F)
    Qt = jnp.einsum('bnhd,hdj->bnhj', Q, WKh)
    logits = jnp.einsum('bnhj,bnkj->bnhk', Qt, h_E) * (1.0 / np.sqrt(D))
    m = mask[:, :, None, :]
    logits = jnp.where(m > 0, logits, NEG_INF)
    att = jax.nn.softmax(logits, axis=-1) * m
    agg = jnp.einsum('bnhk,bnkj->bnhj', att, h_E)
    WVh = W_V.reshape(H, D, F)
    ctx = jnp.einsum('bnhj,hdj->bnhd', agg, WVh).reshape(*h_V.shape[:2], HID)
    return jnp.einsum('bni,oi->bno', ctx, W_O)


def _get_compiled():
    if 'fn' in _CACHE:
        return _CACHE['fn']
    devs = jax.devices()[:8]
    mesh = Mesh(np.asarray(devs), ('x',))
    pn = P(None, 'x')
    fn = jax.jit(shard_map(
        _core_fn, mesh=mesh,
        in_specs=(pn, pn, pn, P(), P(), P(), P()),
        out_specs=pn, check_rep=False))
    _CACHE['fn'] = fn
    return fn


def kernel(h_V, h_E, mask_attend, W_Q, W_K, W_V, W_O):
    fn = _get_compiled()
    out = fn(jnp.asarray(h_V), jnp.asarray(h_E), jnp.asarray(mask_attend),
             jnp.asarray(W_Q), jnp.asarray(W_K), jnp.asarray(W_V), jnp.asarray(W_O))
    return np.asarray(jax.block_until_ready(out)).astype(np.float32)

